# revision 1
# baseline (speedup 1.0000x reference)
"""nn_HashMapper Trainium2 kernel (8 NeuronCores, Bass/Tile).

Contract: kernel(**inputs) takes the FULL unsharded inputs
(bits [32768,1024] i32, tables [3,1024,16384] f32, positions [3,14] i32)
and returns the FULL output [32768,1024] u8.

Sharding (hardcoded): neurons j (1024) are split across the 8 cores (128
each) so the tables are read exactly once system-wide; the batch is split
across cores for address computation and the tiny address tensor
([32768,3] i16) is AllGather'd on-chip.

Per core:
  P0/P1: bits i32 -> bf16 -> PE-transpose k-chunks; PE matmul vs W [1024,3]
      (W[k,h] = 2^(13-kk) at k = 1023-positions[h,kk]) -> addresses
  P2: addresses f32 -> i16 -> DRAM
  P3: AllGather addresses (8 cores)
  P4: wrapped idx loads [128, B/16] per hash (dma_gather index layout)
  P5: table slice [3,128j,16384] f32 -> bf16 -> PE transpose
      -> tT [3,16384,128j] in DRAM
  P6: dma_gather of 256B rows tT[h, addr[b,h], :] (4 SWDGE queues)
  P7: votes = g0+g1+g2; out = votes > 1.5 (u8); writeback [32768, 128]
Host reassembles by concatenating the per-core j-slices.
"""

from contextlib import ExitStack

import numpy as np

import concourse.bass as bass
import concourse.bacc as bacc
import concourse.tile as tile
import concourse.mybir as mybir
from concourse.masks import make_identity
from concourse.bass_utils import run_bass_kernel_spmd

F32 = mybir.dt.float32
BF16 = mybir.dt.bfloat16
I32 = mybir.dt.int32
I16 = mybir.dt.int16
U8 = mybir.dt.uint8

N_BITS = 1024
NE = 16384
H = 3
JS = 128
B_TOTAL = 32768
N_CORES = 8


def _build(b_total=B_TOTAL, n_cores=N_CORES, chunk=2048, nq=4, slots=4):
    bsh = b_total // n_cores  # batch shard per core
    nbt = bsh // 128  # b-tiles in shard
    nck = b_total // chunk  # gather chunks per hash
    use_cc = n_cores > 1

    nc = bacc.Bacc(
        "TRN2", target_bir_lowering=False, num_devices=n_cores, num_swdge_queues=nq
    )
    bits = nc.dram_tensor("bits", [bsh, N_BITS], I32, kind="ExternalInput")
    tslice = nc.dram_tensor("tslice", [H, JS, NE], F32, kind="ExternalInput")
    w = nc.dram_tensor("w", [N_BITS, H], BF16, kind="ExternalInput")
    out = nc.dram_tensor("out", [b_total, JS], U8, kind="ExternalOutput")

    addr_loc = nc.dram_tensor("addr_loc", [bsh, H], I16)
    addr_all = nc.dram_tensor("addr_all", [b_total, H], I16) if use_cc else addr_loc
    tT = nc.dram_tensor("tT", [H, NE, JS], BF16)

    with tile.TileContext(nc) as tc, ExitStack() as ctx:
        const = ctx.enter_context(tc.tile_pool(name="const", bufs=1))
        ps = ctx.enter_context(tc.tile_pool(name="ps", bufs=4, space="PSUM"))
        psaddr = ctx.enter_context(tc.tile_pool(name="psaddr", bufs=4, space="PSUM"))
        ctxA = ExitStack()
        sb = ctxA.enter_context(tc.tile_pool(name="pA", bufs=2))

        # ---- P0+P1: bits -> bf16 -> PE-transpose into trs (no DRAM bounce) ----
        wsb = const.tile([128, 8, H], BF16)
        nc.sync.dma_start(wsb[:, :, :], w.rearrange("(kc p) h -> p kc h", p=128))
        ident = const.tile([128, 128], BF16)
        make_identity(nc, ident[:, :])
        trs = [sb.tile([128, bsh], BF16, tag=f"tr{kc}", bufs=1, name=f"tr{kc}")
               for kc in range(8)]
        for bt in range(nbt):
            t32 = sb.tile([128, N_BITS], I32, tag="bits32")
            nc.sync.dma_start(t32[:], bits[bt * 128 : (bt + 1) * 128, :])
            tbf = sb.tile([128, N_BITS], BF16, tag="bitsbf")
            nc.vector.tensor_copy(tbf[:], t32[:])
            for kc in range(8):
                pb = ps.tile([128, 128], BF16, tag="trps")
                nc.tensor.transpose(
                    pb[:, :], tbf[:, kc * 128 : (kc + 1) * 128], ident[:, :]
                )
                nc.scalar.activation(
                    trs[kc][:, bt * 128 : (bt + 1) * 128],
                    pb[:, :],
                    mybir.ActivationFunctionType.Copy,
                )
        # ---- P2 staging ----
        addr_sb = const.tile([128, nbt, H], I16)
        for bt in range(nbt):
            p = psaddr.tile([128, H], F32, tag="addr")
            for kc in range(8):
                nc.tensor.matmul(
                    p[:, :],
                    trs[kc][:, bt * 128 : (bt + 1) * 128],
                    wsb[:, kc, :],
                    start=(kc == 0),
                    stop=(kc == 7),
                )
            nc.vector.tensor_copy(addr_sb[:, bt, :], p[:, :])
        nc.sync.dma_start(
            addr_loc.rearrange("(bt p) h -> p bt h", p=128), addr_sb[:, :, :]
        )

        # ---- P5: table slice -> bf16 -> transpose -> tT ----
        GRP = 16
        for h in range(H):
            tsl = sb.tile([128, NE], BF16, tag="tsl", bufs=1)
            nc.gpsimd.dma_start(tsl[:], tslice[h, :, :])  # f32 -> bf16 cast
            for g in range(NE // 128 // GRP):
                stage = sb.tile([128, GRP, 128], BF16, tag="stage")
                for t in range(GRP):
                    at = g * GRP + t
                    pt = ps.tile([128, 128], BF16, tag="trps")
                    nc.tensor.transpose(
                        pt[:, :], tsl[:, at * 128 : (at + 1) * 128], ident[:, :]
                    )
                    nc.scalar.activation(
                        stage[:, t, :], pt[:, :], mybir.ActivationFunctionType.Copy
                    )
                dst = tT[h, g * GRP * 128 : (g + 1) * GRP * 128, :].rearrange(
                    "(t a) j -> a t j", a=128
                )
                nc.sync.dma_start(dst, stage[:, :, :])

        # ---- P3: AllGather ----
        if use_cc:
            nc.gpsimd.collective_compute(
                "AllGather",
                mybir.AluOpType.bypass,
                replica_groups=[list(range(n_cores))],
                ins=[addr_loc.ap().opt()],
                outs=[addr_all.ap().opt()],
            )

        ctxA.close()
        sb = ctx.enter_context(tc.tile_pool(name="pB", bufs=2))

        # ---- P4: wrapped idx loads ----
        idxs = []
        ncols = b_total // 16
        for h in range(H):
            it = const.tile([128, ncols], I16, tag=f"idx{h}")
            src = bass.AP(addr_all, h, [[H, 16], [16 * H, ncols]])
            nc.sync.dma_start(it[0:16, :], src)
            for r in (16, 32, 64):
                nc.sync.dma_start(it[r : 2 * r, :], it[0:r, :])
            idxs.append(it)

        # ---- P6+P7: gather + votes + writeback ----
        # Hand-synchronized: per-queue DMA-completion sems make 4 SWDGE
        # queues safe (Tile's auto DMASW lanes are queue-agnostic and could
        # mix completions from different queues into one wait target).
        CC = chunk // 128
        gts = [[sb.tile([128, CC, JS], BF16, tag=f"g{h}s{sl}", bufs=1, name=f"g{h}s{sl}")
                for sl in range(slots)] for h in range(H)]
        ots = [sb.tile([128, CC, JS], U8, tag=f"os{sl}", bufs=1, name=f"os{sl}")
               for sl in range(slots)]
        gsem = [[nc.alloc_semaphore(f"gs{k}_{h}") for h in range(H)] for k in range(nck)]
        vdone = nc.alloc_semaphore("vdone")
        vc = nc.alloc_semaphore("vc")
        osem = nc.alloc_semaphore("osem")
        outv = out.rearrange("(k cc p) j -> k p cc j", p=128, cc=CC)
        with tc.tile_critical():
            # gpsimd stream: issue gathers; per-gather sems, no issue stalls
            for k in range(nck):
                if k >= slots:
                    nc.gpsimd.wait_ge(vdone, k - slots + 1)
                for h in range(H):
                    q = (k * H + h) % nq
                    nc.gpsimd.dma_gather(
                        gts[h][k % slots][:, :, :],
                        tT[h, :, :],
                        idxs[h][:, k * (chunk // 16) : (k + 1) * (chunk // 16)],
                        num_idxs=chunk,
                        num_idxs_reg=chunk,
                        elem_size=JS,
                        single_packet=False,
                        queue_num=q,
                    ).then_inc(gsem[k][h], 16)
            # vector stream: votes + compare
            for k in range(nck):
                for h in range(H):
                    nc.vector.wait_ge(gsem[k][h], 16)
                if k >= slots:
                    nc.vector.wait_ge(osem, 16 * (k - slots + 1))
                g0, g1, g2 = (gts[h][k % slots] for h in range(H))
                nc.vector.tensor_add(g0[:], g0[:], g1[:]).then_inc(vc, 1)
                nc.vector.wait_ge(vc, 2 * k + 1)
                nc.vector.tensor_add(g0[:], g0[:], g2[:]).then_inc(vc, 1)
                nc.vector.wait_ge(vc, 2 * k + 2)
                nc.vector.tensor_scalar(
                    ots[k % slots][:], g0[:], 1.5, None, op0=mybir.AluOpType.is_ge
                ).then_inc(vdone, 1)
            # sync stream: output DMAs
            for k in range(nck):
                nc.sync.wait_ge(vdone, k + 1)
                nc.sync.dma_start(outv[k], ots[k % slots][:, :, :]).then_inc(osem, 16)
            nc.sync.wait_ge(osem, 16 * nck)

    nc.compile()
    return nc


def _make_w(positions):
    import ml_dtypes

    w = np.zeros((N_BITS, H), dtype=np.float32)
    for h in range(H):
        for kk in range(14):
            w[N_BITS - 1 - positions[h, kk], h] += 2.0 ** (13 - kk)
    return w.astype(ml_dtypes.bfloat16)


_NC_CACHE = {}


def _get_nc():
    if "nc" not in _NC_CACHE:
        _NC_CACHE["nc"] = _build()
    return _NC_CACHE["nc"]


def kernel(bits, tables, positions):
    bits = np.ascontiguousarray(np.asarray(bits, dtype=np.int32))
    tables = np.ascontiguousarray(np.asarray(tables, dtype=np.float32))
    positions = np.asarray(positions, dtype=np.int32)

    nc = _get_nc()
    wnp = _make_w(positions)
    bsh = B_TOTAL // N_CORES
    in_maps = [
        {
            "bits": np.ascontiguousarray(bits[c * bsh : (c + 1) * bsh]),
            "tslice": np.ascontiguousarray(tables[:, c * JS : (c + 1) * JS, :]),
            "w": wnp,
        }
        for c in range(N_CORES)
    ]
    res = run_bass_kernel_spmd(nc, in_maps, core_ids=list(range(N_CORES)))
    return np.concatenate([r["out"] for r in res.results], axis=1)



# revision 20
# speedup vs baseline: 16.6739x; 16.6739x over previous
"""nn_HashMapper Trainium2 kernel (8 NeuronCores, Bass/Tile).

Contract: kernel(**inputs) takes the FULL unsharded inputs
(bits [32768,1024] i32, tables [3,1024,16384] f32, positions [3,14] i32)
and returns the FULL output [32768,1024] u8.

Sharding (hardcoded): pure data-parallel over batch — each core gets a
4096-row batch shard; the 3 hash tables are replicated per core, bit-packed
4 neuron-columns per byte in address-major layout [3, 2^14, 256] (one
gathered address = one contiguous 256B row). No cross-core communication.

Host-side prep (layout/dtype transforms of full tensors only):
  - tabp  = tables as bits: tabp[h, a, p] has neuron j = p + 256*i of
    address a at bit i (i in 0..3)
  - bitsw = bits as u8, per-core wrapped to [1024*16, 256]:
    bitsw[k*16 + r, q] = bits[q*16 + r, k] — so the address matmul
    emits addresses directly in the SWDGE gather-index "wrapped
    16-partition" layout (no DRAM round trip, no partition replication)
  - bidx  = gather indices for the 42 needed bit-rows (16 sub-rows
    each), pre-replicated to 128 partitions; pad slots point at row 0
  - wdiag = constant [128, 2, 128] bf16 weights, 16-replicated along
    the output dim so addresses land on all 128 partitions:
    wdiag[(kk%8)*16 + r, kk//8, p] = delta(r == p%16) * 2^(13-kk)

Per core device program:
  P0: ONE dma_gather pulls the 3x14 bit-row slabs (256B sub-rows) into
      [128, 6, 256] — hash h occupies q=2h (kk 0..7, 128 partitions)
      and q=2h+1 (kk 8..13, 96 partitions); u8 -> bf16
  P1: per hash: 2 accumulating matmuls ([128,128]^T x [128,256]) ->
      PSUM [128, 256] f32 = addresses in wrapped layout on every
      partition; -> i16 idx tile (no replication DMAs needed)
  P2: dma_gather of 256B packed rows tabp[h, addr, :] (4 SWDGE queues,
      2 chunks x 2048 idxs x 3 hashes)
  P3: bitwise majority on packed i32 lanes: maj = (g0&g1)|((g0|g1)&g2)
      (bitwise only — DVE int adds route through f32 and would round
      packed lanes); then 4 fused (maj >> i) & 0x01010101 unpack ops,
      each writing a contiguous 256-column slice; u8 writeback
Host reassembles by concatenating the per-core batch shards.
"""

from contextlib import ExitStack

import numpy as np

import concourse.bass as bass
import concourse.bacc as bacc
import concourse.tile as tile
import concourse.mybir as mybir
from concourse.bass_utils import run_bass_kernel_spmd

F32 = mybir.dt.float32
BF16 = mybir.dt.bfloat16
I32 = mybir.dt.int32
I16 = mybir.dt.int16
U8 = mybir.dt.uint8

N_BITS = 1024
NE = 16384
H = 3
K_BITS = 14
B_TOTAL = 32768
N_CORES = 8
BSH = B_TOTAL // N_CORES  # 4096 batch rows per core
WRAP = 16  # SWDGE index-tile partition wrap
NCOL = BSH // WRAP  # 256 index columns per hash
PK = 4  # table columns packed per byte
PB = N_BITS // PK  # 256 packed bytes per table row
CH = 1024  # gather chunk: idxs per dma_gather
NCK = BSH // CH  # 4 chunks
CC = CH // 128  # 8 gathered rows per partition per chunk
SLOTS = 3
NQ = 4
NSLAB = 768  # 3 hashes x 256 slab-row slots (224 used + 32 pad each)


def _build(positions, _phases=("addr", "gather", "vote", "out")):
    """Build the per-core SPMD program. positions ride in as input data;
    `_phases` exists only for local timing attribution experiments."""
    nc = bacc.Bacc(
        "TRN2", target_bir_lowering=False, num_devices=N_CORES, num_swdge_queues=NQ
    )
    bitsw = nc.dram_tensor("bitsw", [N_BITS * WRAP, NCOL], U8, kind="ExternalInput")
    tabp = nc.dram_tensor("tabp", [H, NE, PB], U8, kind="ExternalInput")
    wdiag = nc.dram_tensor("wdiag", [128, 2, 128], BF16, kind="ExternalInput")
    bidx = nc.dram_tensor("bidx", [128, NSLAB // WRAP], I16, kind="ExternalInput")
    out = nc.dram_tensor("out", [BSH, N_BITS], U8, kind="ExternalOutput")

    with tile.TileContext(nc) as tc, ExitStack() as ctx:
        const = ctx.enter_context(tc.tile_pool(name="const", bufs=1))
        ps = ctx.enter_context(tc.tile_pool(name="ps", bufs=4, space="PSUM"))
        sb = ctx.enter_context(tc.tile_pool(name="sb", bufs=2))

        wd = const.tile([128, 2, 128], BF16)
        nc.sync.dma_start(wd[:, :, :], wdiag[:, :, :])
        bx = const.tile([128, NSLAB // WRAP], I16)
        nc.sync.dma_start(bx[:, :], bidx[:, :])

        # ---- P0: one gather pulls all bit-row slabs ----
        bu8 = const.tile([128, NSLAB // 128, NCOL], U8)
        nc.gpsimd.dma_gather(
            bu8[:, :, :],
            bitsw[:, :],
            bx[:, :],
            num_idxs=NSLAB,
            num_idxs_reg=NSLAB,
            elem_size=NCOL,
            single_packet=False,
            queue_num=0,
        )
        bbf = const.tile([128, NSLAB // 128, NCOL], BF16)
        nc.vector.tensor_copy(bbf[:, :, :], bu8[:, :, :])

        # ---- P1: matmul -> wrapped addresses on all 128 partitions ----
        it_all = const.tile([128, H, NCOL], I16)
        for h in range(H):
            p = ps.tile([128, NCOL], F32, tag="addr")
            nc.tensor.matmul(
                p[:, :], wd[:, 0, :], bbf[:, 2 * h, :], start=True, stop=False
            )
            nc.tensor.matmul(
                p[:, :], wd[0:96, 1, :], bbf[0:96, 2 * h + 1, :],
                start=False, stop=True,
            )
            nc.vector.tensor_copy(it_all[:, h, :], p[:, :])

        # ---- P2+P3: gather + majority + unpack + writeback ----
        # Hand-synchronized (as in the proven baseline): per-gather sems make
        # multiple SWDGE queues safe; Tile's auto DMASW lanes are
        # queue-agnostic and could mix completions across queues.
        gts = [
            [
                sb.tile([128, CC, PB], U8, tag=f"g{h}s{s}", bufs=1, name=f"g{h}s{s}")
                for s in range(SLOTS)
            ]
            for h in range(H)
        ]
        ots = [
            sb.tile([128, CC, N_BITS], U8, tag=f"os{s}", bufs=1, name=f"os{s}")
            for s in range(SLOTS)
        ]
        # exact-bit AND mask (0x01010101 can't ride as an op immediate: it
        # exceeds f32's 24-bit mantissa and byte 0 would round away)
        mask32 = const.tile([128, CC, PB // 4], I32)
        nc.vector.memset(mask32[:, :, :], 0x01010101)
        # shift amounts as per-partition scalar APs: the immediate path
        # encodes f32 ImmVals, which the walrus verifier rejects for bitvec
        # ops on i32 operands.
        shc = const.tile([128, PK], I32)
        for i in range(PK):
            nc.vector.memset(shc[:, i : i + 1], i)
        gsem = [[nc.alloc_semaphore(f"gs{k}_{h}") for h in range(H)] for k in range(NCK)]
        vdone = nc.alloc_semaphore("vdone")
        vc = nc.alloc_semaphore("vc")
        osem = [nc.alloc_semaphore(f"osem{s}") for s in range(SLOTS)]
        outv = out.rearrange("(k q p) j -> k p q j", p=128, q=CC)
        do_gather = "gather" in _phases
        do_vote = "vote" in _phases
        do_out = "out" in _phases
        AND, OR = mybir.AluOpType.bitwise_and, mybir.AluOpType.bitwise_or
        SHR = mybir.AluOpType.logical_shift_right
        with tc.tile_critical():
            # gpsimd stream: issue gathers
            for k in range(NCK if do_gather else 0):
                if k >= SLOTS and do_vote:
                    nc.gpsimd.wait_ge(vdone, k - SLOTS + 1)
                for h in range(H):
                    q = (k * H + h) % NQ
                    nc.gpsimd.dma_gather(
                        gts[h][k % SLOTS][:, :, :],
                        tabp[h, :, :],
                        it_all[:, h, k * (CH // WRAP) : (k + 1) * (CH // WRAP)],
                        num_idxs=CH,
                        num_idxs_reg=CH,
                        elem_size=PB,
                        single_packet=False,
                        queue_num=q,
                    ).then_inc(gsem[k][h], 16)
            # vector stream: bitwise majority on packed lanes, then unpack.
            for k in range(NCK if do_vote else 0):
                for h in range(H):
                    nc.vector.wait_ge(gsem[k][h], 16)
                if k >= SLOTS and do_out:
                    nc.vector.wait_ge(osem[k % SLOTS], 16 * (k // SLOTS))
                g0, g1, g2 = (
                    gts[h][k % SLOTS][:, :, :].bitcast(I32) for h in range(H)
                )
                ot32 = ots[k % SLOTS][:, :, :].bitcast(I32)
                # xor-median: maj = g1 ^ ((g1^g0) & (g1^g2)), kept in g1
                XOR = mybir.AluOpType.bitwise_xor
                nc.vector.tensor_tensor(g0, g0, g1, op=XOR).then_inc(vc, 1)
                nc.vector.tensor_tensor(g2, g2, g1, op=XOR).then_inc(vc, 1)
                nc.vector.wait_ge(vc, 7 * k + 2)
                nc.vector.tensor_tensor(g0, g0, g2, op=AND).then_inc(vc, 1)
                nc.vector.wait_ge(vc, 7 * k + 3)
                nc.vector.tensor_tensor(g1, g1, g0, op=XOR).then_inc(vc, 1)
                nc.vector.wait_ge(vc, 7 * k + 4)
                # unpack bit i -> contiguous 256-column slice (i32 view);
                # the 4 unpacks are independent (disjoint out slices)
                for i in range(PK):
                    nc.vector.scalar_tensor_tensor(
                        ot32[:, :, i * (PB // 4) : (i + 1) * (PB // 4)],
                        g1,
                        shc[:, i : i + 1],
                        mask32[:, :, :],
                        op0=SHR,
                        op1=AND,
                    ).then_inc(vdone if i == PK - 1 else vc, 1)
            # sync stream: output DMAs
            for k in range(NCK if do_out else 0):
                nc.sync.wait_ge(vdone, k + 1)
                nc.sync.dma_start(outv[k], ots[k % SLOTS][:, :, :]).then_inc(
                    osem[k % SLOTS], 16
                )
            if do_out:
                for s in range(SLOTS):
                    nc.sync.wait_ge(osem[s], 16 * (NCK // SLOTS))

    nc.compile()
    return nc


def _make_wdiag():
    import ml_dtypes

    wd = np.zeros((128, 2, 128), np.float32)
    for kk in range(K_BITS):
        for r in range(WRAP):
            for p in range(r, 128, WRAP):
                wd[(kk % 8) * WRAP + r, kk // 8, p] = 2.0 ** (13 - kk)
    return wd.astype(ml_dtypes.bfloat16)


def _make_bidx(positions):
    # slab-row gather indices: slot i -> (h = i//256, j = i%256);
    # j < 224 -> bitsw row (1023 - positions[h, j//16]) * 16 + (j%16);
    # pad slots -> row 0 (harmlessly gathered, never read).
    rows = N_BITS - 1 - np.asarray(positions, np.int64)  # [H, K_BITS]
    assert rows.shape == (H, K_BITS) and rows.min() >= 0 and rows.max() < N_BITS
    idx = np.zeros(NSLAB, np.int16)
    for h in range(H):
        for kk in range(K_BITS):
            for r in range(WRAP):
                idx[h * 256 + kk * WRAP + r] = rows[h, kk] * WRAP + r
    wrapped = idx.reshape(NSLAB // WRAP, WRAP).T  # [16, 48]
    return np.ascontiguousarray(np.tile(wrapped, (8, 1)))  # [128, 48]


_NC_CACHE = {}


def _get_nc(positions):
    if "nc" not in _NC_CACHE:
        _NC_CACHE["nc"] = _build(positions)
    return _NC_CACHE["nc"]


def _prep_tables(tables):
    # binary f32 -> bit-packed u8, address-major: tabp[h, a, p] bit i holds
    # tables[h, p + PB*i, a]
    t8 = np.asarray(tables, np.float32).astype(np.uint8)  # [H, N_BITS, NE]
    tT = np.ascontiguousarray(t8.transpose(0, 2, 1))  # [H, NE, N_BITS]
    tp = np.zeros((H, NE, PB), np.uint8)
    for i in range(PK):
        tp |= tT[:, :, i * PB : (i + 1) * PB] << i
    return tp


def _prep_bits(bits):
    # per-core wrapped layout: bw[c, k*16 + r, q] = bits[c*BSH + q*16 + r, k]
    b8 = np.asarray(bits, np.int32).astype(np.uint8)
    bw = b8.reshape(N_CORES, NCOL, WRAP, N_BITS)
    return np.ascontiguousarray(bw.transpose(0, 3, 2, 1)).reshape(
        N_CORES, N_BITS * WRAP, NCOL
    )


def kernel(bits, tables, positions):
    positions = np.asarray(positions, np.int32)
    nc = _get_nc(positions)
    tabp = _prep_tables(tables)
    bw = _prep_bits(bits)
    wd = _make_wdiag()
    bx = _make_bidx(positions)
    in_maps = [
        {"bitsw": bw[c], "tabp": tabp, "wdiag": wd, "bidx": bx}
        for c in range(N_CORES)
    ]
    res = run_bass_kernel_spmd(nc, in_maps, core_ids=list(range(N_CORES)))
    return np.concatenate([r["out"] for r in res.results], axis=0)


# revision 26
# speedup vs baseline: 19.6855x; 1.1806x over previous
"""nn_HashMapper Trainium2 kernel (8 NeuronCores, Bass/Tile).

Contract: kernel(**inputs) takes the FULL unsharded inputs
(bits [32768,1024] i32, tables [3,1024,16384] f32, positions [3,14] i32)
and returns the FULL output [32768,1024] u8.

Sharding (hardcoded): pure data-parallel over batch — each core gets a
4096-row batch shard; the 3 hash tables are replicated per core, bit-packed
4 neuron-columns per byte in address-major layout [3, 2^14, 256] (one
gathered address = one contiguous 256B row). No cross-core communication.

Host-side prep (layout/dtype transforms of full tensors only):
  - tabp  = tables as bits: tabp[h, a, p] has neuron j = p + 256*i of
    address a at bit i (i in 0..3)
  - bitsw = bits as bf16, per-core wrapped to [1024*16, 256]:
    bitsw[k*16 + r, q] = bits[q*16 + r, k] — so the address matmul
    emits addresses directly in the SWDGE gather-index "wrapped
    16-partition" layout (no DRAM round trip, no partition replication)
  - bidx  = gather indices for the 42 needed bit-rows (16 sub-rows
    each), pre-replicated to 128 partitions; pad slots point at row 0
  - wdiag = constant [128, 2, 128] bf16 weights, 16-replicated along
    the output dim so addresses land on all 128 partitions:
    wdiag[(kk%8)*16 + r, kk//8, p] = delta(r == p%16) * 2^(13-kk)

Per core device program:
  P0: ONE dma_gather pulls the 3x14 bit-row slabs (256B sub-rows) into
      [128, 6, 256] bf16 — hash h occupies q=2h (kk 0..7, 128
      partitions) and q=2h+1 (kk 8..13, 96 partitions); matmul-ready
  P1: per hash: 2 accumulating matmuls ([128,128]^T x [128,256]) ->
      PSUM [128, 256] f32 = addresses in wrapped layout on every
      partition; -> i16 idx tile (no replication DMAs needed)
  P2: dma_gather of 256B packed rows tabp[h, addr, :] (4 SWDGE queues,
      5 chunks x 3 hashes; tapered chunk sizes shorten the exposed tail)
  P3: bitwise majority on packed i32 lanes (xor-median form; bitwise
      only — DVE int adds route through f32 and would round packed
      lanes); then 4 fused (maj >> i) & 0x01010101 unpack ops,
      each writing a contiguous 256-column slice; u8 writeback
Host reassembles by concatenating the per-core batch shards.
"""

from contextlib import ExitStack

import numpy as np

import concourse.bass as bass
import concourse.bacc as bacc
import concourse.tile as tile
import concourse.mybir as mybir
from concourse.bass_utils import run_bass_kernel_spmd

F32 = mybir.dt.float32
BF16 = mybir.dt.bfloat16
I32 = mybir.dt.int32
I16 = mybir.dt.int16
U8 = mybir.dt.uint8

N_BITS = 1024
NE = 16384
H = 3
K_BITS = 14
B_TOTAL = 32768
N_CORES = 8
BSH = B_TOTAL // N_CORES  # 4096 batch rows per core
WRAP = 16  # SWDGE index-tile partition wrap
NCOL = BSH // WRAP  # 256 index columns per hash
PK = 4  # table columns packed per byte
PB = N_BITS // PK  # 256 packed bytes per table row
PLAN = (1024, 1024, 1024, 512, 512)  # gather chunk sizes (batch rows)
SLOTS = 4
NQ = 4
NSLAB = 768  # 3 hashes x 256 slab-row slots (224 used + 32 pad each)


def _build(positions, _phases=("addr", "gather", "vote", "out"), _plan=PLAN, _slots=SLOTS):
    """Build the per-core SPMD program. positions ride in as input data;
    `_phases`/`_plan`/`_slots` exist only for local timing experiments."""
    SLOTS = _slots
    plan = list(_plan)  # chunk lengths in batch rows
    assert sum(plan) == BSH and all(ln % 128 == 0 for ln in plan)
    offs = np.cumsum([0] + plan[:-1]).tolist()
    NCK = len(plan)
    CC = max(plan) // 128
    nc = bacc.Bacc(
        "TRN2", target_bir_lowering=False, num_devices=N_CORES, num_swdge_queues=NQ
    )
    bitsw = nc.dram_tensor("bitsw", [N_BITS * WRAP, NCOL], BF16, kind="ExternalInput")
    tabp = nc.dram_tensor("tabp", [H, NE, PB], U8, kind="ExternalInput")
    wdiag = nc.dram_tensor("wdiag", [128, 2, 128], BF16, kind="ExternalInput")
    bidx = nc.dram_tensor("bidx", [128, NSLAB // WRAP], I16, kind="ExternalInput")
    out = nc.dram_tensor("out", [BSH, N_BITS], U8, kind="ExternalOutput")

    with tile.TileContext(nc) as tc, ExitStack() as ctx:
        const = ctx.enter_context(tc.tile_pool(name="const", bufs=1))
        ps = ctx.enter_context(tc.tile_pool(name="ps", bufs=4, space="PSUM"))
        sb = ctx.enter_context(tc.tile_pool(name="sb", bufs=2))

        bx = const.tile([128, NSLAB // WRAP], I16)
        nc.sync.dma_start(bx[:, :], bidx[:, :])
        wd = const.tile([128, 2, 128], BF16)
        nc.sync.dma_start(wd[:, :, :], wdiag[:, :, :])

        # ---- P0: one gather pulls all bit-row slabs (bf16, matmul-ready) ----
        bbf = const.tile([128, NSLAB // 128, NCOL], BF16)
        nc.gpsimd.dma_gather(
            bbf[:, :, :],
            bitsw[:, :],
            bx[:, :],
            num_idxs=NSLAB,
            num_idxs_reg=NSLAB,
            elem_size=NCOL,
            single_packet=False,
            queue_num=0,
        )

        # ---- P1: matmul -> wrapped addresses on all 128 partitions ----
        it_all = const.tile([128, H, NCOL], I16)
        for h in range(H):
            p = ps.tile([128, NCOL], F32, tag="addr")
            nc.tensor.matmul(
                p[:, :], wd[:, 0, :], bbf[:, 2 * h, :], start=True, stop=False
            )
            nc.tensor.matmul(
                p[:, :], wd[0:96, 1, :], bbf[0:96, 2 * h + 1, :],
                start=False, stop=True,
            )
            nc.vector.tensor_copy(it_all[:, h, :], p[:, :])

        # ---- P2+P3: gather + majority + unpack + writeback ----
        # Hand-synchronized (as in the proven baseline): per-gather sems make
        # multiple SWDGE queues safe; Tile's auto DMASW lanes are
        # queue-agnostic and could mix completions across queues.
        gts = [
            [
                sb.tile([128, CC, PB], U8, tag=f"g{h}s{s}", bufs=1, name=f"g{h}s{s}")
                for s in range(SLOTS)
            ]
            for h in range(H)
        ]
        ots = [
            sb.tile([128, CC, N_BITS], U8, tag=f"os{s}", bufs=1, name=f"os{s}")
            for s in range(SLOTS)
        ]
        # exact-bit AND mask (0x01010101 can't ride as an op immediate: it
        # exceeds f32's 24-bit mantissa and byte 0 would round away)
        mask32 = const.tile([128, CC, PB // 4], I32)
        nc.vector.memset(mask32[:, :, :], 0x01010101)
        # shift amounts as per-partition scalar APs: the immediate path
        # encodes f32 ImmVals, which the walrus verifier rejects for bitvec
        # ops on i32 operands.
        shc = const.tile([128, PK], I32)
        for i in range(PK):
            nc.vector.memset(shc[:, i : i + 1], i)
        gsem = [[nc.alloc_semaphore(f"gs{k}_{h}") for h in range(H)] for k in range(NCK)]
        vdone = nc.alloc_semaphore("vdone")
        vc = nc.alloc_semaphore("vc")
        osem = [nc.alloc_semaphore(f"osem{s}") for s in range(SLOTS)]
        do_gather = "gather" in _phases
        do_vote = "vote" in _phases
        do_out = "out" in _phases
        AND, OR = mybir.AluOpType.bitwise_and, mybir.AluOpType.bitwise_or
        SHR = mybir.AluOpType.logical_shift_right
        with tc.tile_critical():
            # gpsimd stream: issue gathers
            for k in range(NCK if do_gather else 0):
                off, ln = offs[k], plan[k]
                cck = ln // 128
                if k >= SLOTS and do_vote:
                    nc.gpsimd.wait_ge(vdone, k - SLOTS + 1)
                for h in range(H):
                    q = (k * H + h) % NQ
                    nc.gpsimd.dma_gather(
                        gts[h][k % SLOTS][:, 0:cck, :],
                        tabp[h, :, :],
                        it_all[:, h, off // WRAP : (off + ln) // WRAP],
                        num_idxs=ln,
                        num_idxs_reg=ln,
                        elem_size=PB,
                        single_packet=False,
                        queue_num=q,
                    ).then_inc(gsem[k][h], 16)
            # vector stream: bitwise majority on packed lanes, then unpack.
            for k in range(NCK if do_vote else 0):
                for h in range(H):
                    nc.vector.wait_ge(gsem[k][h], 16)
                if k >= SLOTS and do_out:
                    nc.vector.wait_ge(osem[k % SLOTS], 16 * (k // SLOTS))
                cck = plan[k] // 128
                g0, g1, g2 = (
                    gts[h][k % SLOTS][:, 0:cck, :].bitcast(I32) for h in range(H)
                )
                ot32 = ots[k % SLOTS][:, 0:cck, :].bitcast(I32)
                # xor-median: maj = g1 ^ ((g1^g0) & (g1^g2)), kept in g1
                XOR = mybir.AluOpType.bitwise_xor
                nc.vector.tensor_tensor(g0, g0, g1, op=XOR).then_inc(vc, 1)
                nc.vector.tensor_tensor(g2, g2, g1, op=XOR).then_inc(vc, 1)
                nc.vector.wait_ge(vc, 7 * k + 2)
                nc.vector.tensor_tensor(g0, g0, g2, op=AND).then_inc(vc, 1)
                nc.vector.wait_ge(vc, 7 * k + 3)
                nc.vector.tensor_tensor(g1, g1, g0, op=XOR).then_inc(vc, 1)
                nc.vector.wait_ge(vc, 7 * k + 4)
                # unpack bit i -> contiguous 256-column slice (i32 view);
                # the 4 unpacks are independent (disjoint out slices)
                for i in range(PK):
                    nc.vector.scalar_tensor_tensor(
                        ot32[:, :, i * (PB // 4) : (i + 1) * (PB // 4)],
                        g1,
                        shc[:, i : i + 1],
                        mask32[:, 0:cck, :],
                        op0=SHR,
                        op1=AND,
                    ).then_inc(vdone if i == PK - 1 else vc, 1)
            # sync stream: output DMAs
            for k in range(NCK if do_out else 0):
                off, ln = offs[k], plan[k]
                dst = out[off : off + ln, :].rearrange("(q p) j -> p q j", p=128)
                nc.sync.wait_ge(vdone, k + 1)
                nc.sync.dma_start(dst, ots[k % SLOTS][:, 0 : ln // 128, :]).then_inc(
                    osem[k % SLOTS], 16
                )
            if do_out:
                for s in range(SLOTS):
                    uses = len([k for k in range(NCK) if k % SLOTS == s])
                    nc.sync.wait_ge(osem[s], 16 * uses)

    nc.compile()
    return nc


def _make_wdiag():
    import ml_dtypes

    wd = np.zeros((128, 2, 128), np.float32)
    for kk in range(K_BITS):
        for r in range(WRAP):
            for p in range(r, 128, WRAP):
                wd[(kk % 8) * WRAP + r, kk // 8, p] = 2.0 ** (13 - kk)
    return wd.astype(ml_dtypes.bfloat16)


def _make_bidx(positions):
    # slab-row gather indices: slot i -> (h = i//256, j = i%256);
    # j < 224 -> bitsw row (1023 - positions[h, j//16]) * 16 + (j%16);
    # pad slots -> row 0 (harmlessly gathered, never read).
    rows = N_BITS - 1 - np.asarray(positions, np.int64)  # [H, K_BITS]
    assert rows.shape == (H, K_BITS) and rows.min() >= 0 and rows.max() < N_BITS
    idx = np.zeros(NSLAB, np.int16)
    for h in range(H):
        for kk in range(K_BITS):
            for r in range(WRAP):
                idx[h * 256 + kk * WRAP + r] = rows[h, kk] * WRAP + r
    wrapped = idx.reshape(NSLAB // WRAP, WRAP).T  # [16, 48]
    return np.ascontiguousarray(np.tile(wrapped, (8, 1)))  # [128, 48]


_NC_CACHE = {}


def _get_nc(positions):
    if "nc" not in _NC_CACHE:
        _NC_CACHE["nc"] = _build(positions)
    return _NC_CACHE["nc"]


def _prep_tables(tables):
    # binary f32 -> bit-packed u8, address-major: tabp[h, a, p] bit i holds
    # tables[h, p + PB*i, a]
    t8 = np.asarray(tables, np.float32).astype(np.uint8)  # [H, N_BITS, NE]
    tT = np.ascontiguousarray(t8.transpose(0, 2, 1))  # [H, NE, N_BITS]
    tp = np.zeros((H, NE, PB), np.uint8)
    for i in range(PK):
        tp |= tT[:, :, i * PB : (i + 1) * PB] << i
    return tp


def _prep_bits(bits):
    # per-core wrapped layout: bw[c, k*16 + r, q] = bits[c*BSH + q*16 + r, k]
    # as bf16 so gathered slabs feed the PE matmul without a convert
    import ml_dtypes

    b = np.asarray(bits, np.int32).astype(ml_dtypes.bfloat16)
    bw = b.reshape(N_CORES, NCOL, WRAP, N_BITS)
    return np.ascontiguousarray(bw.transpose(0, 3, 2, 1)).reshape(
        N_CORES, N_BITS * WRAP, NCOL
    )


def kernel(bits, tables, positions):
    positions = np.asarray(positions, np.int32)
    nc = _get_nc(positions)
    tabp = _prep_tables(tables)
    bw = _prep_bits(bits)
    wd = _make_wdiag()
    bx = _make_bidx(positions)
    in_maps = [
        {"bitsw": bw[c], "tabp": tabp, "wdiag": wd, "bidx": bx}
        for c in range(N_CORES)
    ]
    res = run_bass_kernel_spmd(nc, in_maps, core_ids=list(range(N_CORES)))
    return np.concatenate([r["out"] for r in res.results], axis=0)


# revision 27
# speedup vs baseline: 20.1394x; 1.0231x over previous
"""nn_HashMapper Trainium2 kernel (8 NeuronCores, Bass/Tile).

Contract: kernel(**inputs) takes the FULL unsharded inputs
(bits [32768,1024] i32, tables [3,1024,16384] f32, positions [3,14] i32)
and returns the FULL output [32768,1024] u8.

Sharding (hardcoded): pure data-parallel over batch — each core gets a
4096-row batch shard; the 3 hash tables are replicated per core, bit-packed
4 neuron-columns per byte in address-major layout [3, 2^14, 256] (one
gathered address = one contiguous 256B row). No cross-core communication.

Host-side prep (layout/dtype transforms of full tensors only):
  - tabp  = tables as bits: tabp[h, a, p] has neuron j = p + 256*i of
    address a at bit i (i in 0..3)
  - bitsw = bits as bf16, per-core wrapped to [1024*16, 256]:
    bitsw[k*16 + r, q] = bits[q*16 + r, k] — so the address matmul
    emits addresses directly in the SWDGE gather-index "wrapped
    16-partition" layout (no DRAM round trip, no partition replication)
  - bidx  = gather indices for the 42 needed bit-rows (16 sub-rows
    each), pre-replicated to 128 partitions; pad slots point at row 0
  - wdiag = constant [128, 2, 128] bf16 weights, 16-replicated along
    the output dim so addresses land on all 128 partitions:
    wdiag[(kk%8)*16 + r, kk//8, p] = delta(r == p%16) * 2^(13-kk)

Per core device program:
  P0: ONE dma_gather pulls the 3x14 bit-row slabs (256B sub-rows) into
      [128, 6, 256] bf16 — hash h occupies q=2h (kk 0..7, 128
      partitions) and q=2h+1 (kk 8..13, 96 partitions); matmul-ready
  P1: per hash: 2 accumulating matmuls ([128,128]^T x [128,256]) ->
      PSUM [128, 256] f32 = addresses in wrapped layout on every
      partition; -> i16 idx tile (no replication DMAs needed)
  P2: dma_gather of 256B packed rows tabp[h, addr, :] (4 SWDGE queues,
      5 chunks x 3 hashes; tapered chunk sizes shorten the exposed tail)
  P3: bitwise majority on packed i32 lanes (xor-median form; bitwise
      only — DVE int adds route through f32 and would round packed
      lanes); then 4 fused (maj >> i) & 0x01010101 unpack ops,
      each writing a contiguous 256-column slice; u8 writeback
Host reassembles by concatenating the per-core batch shards.
"""

from contextlib import ExitStack

import numpy as np

import concourse.bass as bass
import concourse.bacc as bacc
import concourse.tile as tile
import concourse.mybir as mybir
from concourse.bass_utils import run_bass_kernel_spmd

F32 = mybir.dt.float32
BF16 = mybir.dt.bfloat16
I32 = mybir.dt.int32
I16 = mybir.dt.int16
U8 = mybir.dt.uint8

N_BITS = 1024
NE = 16384
H = 3
K_BITS = 14
B_TOTAL = 32768
N_CORES = 8
BSH = B_TOTAL // N_CORES  # 4096 batch rows per core
WRAP = 16  # SWDGE index-tile partition wrap
NCOL = BSH // WRAP  # 256 index columns per hash
PK = 4  # table columns packed per byte
PB = N_BITS // PK  # 256 packed bytes per table row
PLAN = (1024, 1024, 1024, 768, 256)  # gather chunk sizes (batch rows)
SLOTS = 4
NQ = 4
NSLAB = 768  # 3 hashes x 256 slab-row slots (224 used + 32 pad each)


def _build(positions, _phases=("addr", "gather", "vote", "out"), _plan=PLAN, _slots=SLOTS):
    """Build the per-core SPMD program. positions ride in as input data;
    `_phases`/`_plan`/`_slots` exist only for local timing experiments."""
    SLOTS = _slots
    plan = list(_plan)  # chunk lengths in batch rows
    assert sum(plan) == BSH and all(ln % 128 == 0 for ln in plan)
    offs = np.cumsum([0] + plan[:-1]).tolist()
    NCK = len(plan)
    CC = max(plan) // 128
    nc = bacc.Bacc(
        "TRN2", target_bir_lowering=False, num_devices=N_CORES, num_swdge_queues=NQ
    )
    bitsw = nc.dram_tensor("bitsw", [N_BITS * WRAP, NCOL], BF16, kind="ExternalInput")
    tabp = nc.dram_tensor("tabp", [H, NE, PB], U8, kind="ExternalInput")
    wdiag = nc.dram_tensor("wdiag", [128, 2, 128], BF16, kind="ExternalInput")
    bidx = nc.dram_tensor("bidx", [128, NSLAB // WRAP], I16, kind="ExternalInput")
    out = nc.dram_tensor("out", [BSH, N_BITS], U8, kind="ExternalOutput")

    with tile.TileContext(nc) as tc, ExitStack() as ctx:
        const = ctx.enter_context(tc.tile_pool(name="const", bufs=1))
        ps = ctx.enter_context(tc.tile_pool(name="ps", bufs=4, space="PSUM"))
        sb = ctx.enter_context(tc.tile_pool(name="sb", bufs=2))

        bx = const.tile([128, NSLAB // WRAP], I16)
        nc.sync.dma_start(bx[:, :], bidx[:, :])
        wd = const.tile([128, 2, 128], BF16)
        nc.sync.dma_start(wd[:, :, :], wdiag[:, :, :])

        # ---- P0: one gather pulls all bit-row slabs (bf16, matmul-ready) ----
        bbf = const.tile([128, NSLAB // 128, NCOL], BF16)
        nc.gpsimd.dma_gather(
            bbf[:, :, :],
            bitsw[:, :],
            bx[:, :],
            num_idxs=NSLAB,
            num_idxs_reg=NSLAB,
            elem_size=NCOL,
            single_packet=False,
            queue_num=0,
        )

        # ---- P1: matmul -> wrapped addresses on all 128 partitions ----
        it_all = const.tile([128, H, NCOL], I16)
        for h in range(H):
            p = ps.tile([128, NCOL], F32, tag="addr")
            nc.tensor.matmul(
                p[:, :], wd[:, 0, :], bbf[:, 2 * h, :], start=True, stop=False
            )
            nc.tensor.matmul(
                p[:, :], wd[0:96, 1, :], bbf[0:96, 2 * h + 1, :],
                start=False, stop=True,
            )
            nc.vector.tensor_copy(it_all[:, h, :], p[:, :])

        # ---- P2+P3: gather + majority + unpack + writeback ----
        # Hand-synchronized (as in the proven baseline): per-gather sems make
        # multiple SWDGE queues safe; Tile's auto DMASW lanes are
        # queue-agnostic and could mix completions across queues.
        gts = [
            [
                sb.tile([128, CC, PB], U8, tag=f"g{h}s{s}", bufs=1, name=f"g{h}s{s}")
                for s in range(SLOTS)
            ]
            for h in range(H)
        ]
        ots = [
            sb.tile([128, CC, N_BITS], U8, tag=f"os{s}", bufs=1, name=f"os{s}")
            for s in range(SLOTS)
        ]
        # exact-bit AND mask (0x01010101 can't ride as an op immediate: it
        # exceeds f32's 24-bit mantissa and byte 0 would round away)
        mask32 = const.tile([128, CC, PB // 4], I32)
        nc.vector.memset(mask32[:, :, :], 0x01010101)
        # shift amounts as per-partition scalar APs: the immediate path
        # encodes f32 ImmVals, which the walrus verifier rejects for bitvec
        # ops on i32 operands.
        shc = const.tile([128, PK], I32)
        for i in range(PK):
            nc.vector.memset(shc[:, i : i + 1], i)
        gsem = [[nc.alloc_semaphore(f"gs{k}_{h}") for h in range(H)] for k in range(NCK)]
        vdone = nc.alloc_semaphore("vdone")
        vc = nc.alloc_semaphore("vc")
        osem = [nc.alloc_semaphore(f"osem{s}") for s in range(SLOTS)]
        do_gather = "gather" in _phases
        do_vote = "vote" in _phases
        do_out = "out" in _phases
        AND, OR = mybir.AluOpType.bitwise_and, mybir.AluOpType.bitwise_or
        SHR = mybir.AluOpType.logical_shift_right
        with tc.tile_critical():
            # gpsimd stream: issue gathers
            for k in range(NCK if do_gather else 0):
                off, ln = offs[k], plan[k]
                cck = ln // 128
                if k >= SLOTS and do_vote:
                    nc.gpsimd.wait_ge(vdone, k - SLOTS + 1)
                for h in range(H):
                    q = (k * H + h) % NQ
                    nc.gpsimd.dma_gather(
                        gts[h][k % SLOTS][:, 0:cck, :],
                        tabp[h, :, :],
                        it_all[:, h, off // WRAP : (off + ln) // WRAP],
                        num_idxs=ln,
                        num_idxs_reg=ln,
                        elem_size=PB,
                        single_packet=False,
                        queue_num=q,
                    ).then_inc(gsem[k][h], 16)
            # vector stream: bitwise majority on packed lanes, then unpack.
            for k in range(NCK if do_vote else 0):
                for h in range(H):
                    nc.vector.wait_ge(gsem[k][h], 16)
                if k >= SLOTS and do_out:
                    nc.vector.wait_ge(osem[k % SLOTS], 16 * (k // SLOTS))
                cck = plan[k] // 128
                g0, g1, g2 = (
                    gts[h][k % SLOTS][:, 0:cck, :].bitcast(I32) for h in range(H)
                )
                ot32 = ots[k % SLOTS][:, 0:cck, :].bitcast(I32)
                # xor-median: maj = g1 ^ ((g1^g0) & (g1^g2)), kept in g1
                XOR = mybir.AluOpType.bitwise_xor
                nc.vector.tensor_tensor(g0, g0, g1, op=XOR).then_inc(vc, 1)
                nc.vector.tensor_tensor(g2, g2, g1, op=XOR).then_inc(vc, 1)
                nc.vector.wait_ge(vc, 7 * k + 2)
                nc.vector.tensor_tensor(g0, g0, g2, op=AND).then_inc(vc, 1)
                nc.vector.wait_ge(vc, 7 * k + 3)
                nc.vector.tensor_tensor(g1, g1, g0, op=XOR).then_inc(vc, 1)
                nc.vector.wait_ge(vc, 7 * k + 4)
                # unpack bit i -> contiguous 256-column slice (i32 view);
                # the 4 unpacks are independent (disjoint out slices)
                for i in range(PK):
                    nc.vector.scalar_tensor_tensor(
                        ot32[:, :, i * (PB // 4) : (i + 1) * (PB // 4)],
                        g1,
                        shc[:, i : i + 1],
                        mask32[:, 0:cck, :],
                        op0=SHR,
                        op1=AND,
                    ).then_inc(vdone if i == PK - 1 else vc, 1)
            # sync stream: output DMAs
            for k in range(NCK if do_out else 0):
                off, ln = offs[k], plan[k]
                dst = out[off : off + ln, :].rearrange("(q p) j -> p q j", p=128)
                nc.sync.wait_ge(vdone, k + 1)
                nc.sync.dma_start(dst, ots[k % SLOTS][:, 0 : ln // 128, :]).then_inc(
                    osem[k % SLOTS], 16
                )
            if do_out:
                for s in range(SLOTS):
                    uses = len([k for k in range(NCK) if k % SLOTS == s])
                    nc.sync.wait_ge(osem[s], 16 * uses)

    nc.compile()
    return nc


def _make_wdiag():
    import ml_dtypes

    wd = np.zeros((128, 2, 128), np.float32)
    for kk in range(K_BITS):
        for r in range(WRAP):
            for p in range(r, 128, WRAP):
                wd[(kk % 8) * WRAP + r, kk // 8, p] = 2.0 ** (13 - kk)
    return wd.astype(ml_dtypes.bfloat16)


def _make_bidx(positions):
    # slab-row gather indices: slot i -> (h = i//256, j = i%256);
    # j < 224 -> bitsw row (1023 - positions[h, j//16]) * 16 + (j%16);
    # pad slots -> row 0 (harmlessly gathered, never read).
    rows = N_BITS - 1 - np.asarray(positions, np.int64)  # [H, K_BITS]
    assert rows.shape == (H, K_BITS) and rows.min() >= 0 and rows.max() < N_BITS
    idx = np.zeros(NSLAB, np.int16)
    for h in range(H):
        for kk in range(K_BITS):
            for r in range(WRAP):
                idx[h * 256 + kk * WRAP + r] = rows[h, kk] * WRAP + r
    wrapped = idx.reshape(NSLAB // WRAP, WRAP).T  # [16, 48]
    return np.ascontiguousarray(np.tile(wrapped, (8, 1)))  # [128, 48]


_NC_CACHE = {}


def _get_nc(positions):
    if "nc" not in _NC_CACHE:
        _NC_CACHE["nc"] = _build(positions)
    return _NC_CACHE["nc"]


def _prep_tables(tables):
    # binary f32 -> bit-packed u8, address-major: tabp[h, a, p] bit i holds
    # tables[h, p + PB*i, a]
    t8 = np.asarray(tables, np.float32).astype(np.uint8)  # [H, N_BITS, NE]
    tT = np.ascontiguousarray(t8.transpose(0, 2, 1))  # [H, NE, N_BITS]
    tp = np.zeros((H, NE, PB), np.uint8)
    for i in range(PK):
        tp |= tT[:, :, i * PB : (i + 1) * PB] << i
    return tp


def _prep_bits(bits):
    # per-core wrapped layout: bw[c, k*16 + r, q] = bits[c*BSH + q*16 + r, k]
    # as bf16 so gathered slabs feed the PE matmul without a convert
    import ml_dtypes

    b = np.asarray(bits, np.int32).astype(ml_dtypes.bfloat16)
    bw = b.reshape(N_CORES, NCOL, WRAP, N_BITS)
    return np.ascontiguousarray(bw.transpose(0, 3, 2, 1)).reshape(
        N_CORES, N_BITS * WRAP, NCOL
    )


def kernel(bits, tables, positions):
    positions = np.asarray(positions, np.int32)
    nc = _get_nc(positions)
    tabp = _prep_tables(tables)
    bw = _prep_bits(bits)
    wd = _make_wdiag()
    bx = _make_bidx(positions)
    in_maps = [
        {"bitsw": bw[c], "tabp": tabp, "wdiag": wd, "bidx": bx}
        for c in range(N_CORES)
    ]
    res = run_bass_kernel_spmd(nc, in_maps, core_ids=list(range(N_CORES)))
    return np.concatenate([r["out"] for r in res.results], axis=0)
